# revision 1
# baseline (speedup 1.0000x reference)
"""ContraNorm kernel for 8x Trainium2 NeuronCores (Bass/Tile).

Computes, for x [8192, 512] fp32 (gamma/beta [512]):
    xn  = x / max(||x||_row, eps)
    sim = xn @ xn.T
    sim = softmax(sim, axis=1) + softmax(sim, axis=0)
    y   = x - 0.1 * (sim @ x)
    out = LayerNorm(y) * gamma + beta          (eps = 1e-6)

Key math used by the kernel:
  * sim entries are cosine similarities in [-1, 1], so exp() never
    overflows and softmax needs no max-subtraction:
        row_softmax[i,j] = E[i,j] / r_i,  E = exp(sim),  r_i = sum_j E[i,j]
  * E is symmetric, so column sums equal row sums:  c_j = r_j.
        sim' = E * (1/r_i + 1/r_j)   (elementwise)
  * Row-shard across 8 cores. Core q owns rows [q*1024, (q+1)*1024).
    It computes E^T tiles T[j, i] (j = all 8192 on partitions,
    i = its 1024 rows on free dim); ACT exp accumulates partial row
    sums; one 32KB AllReduce + ReduceScatter of those partials gives
    every core the full r (for 1/r_j, per-partition) and its own slice
    (for 1/r_i, partition-broadcast) without any core-id branching.
"""

import sys

if "/opt/trn_rl_repo" not in sys.path:
    sys.path.insert(0, "/opt/trn_rl_repo")

import ml_dtypes
import numpy as np

import concourse.bass as bass
import concourse.tile as tile
from concourse import bacc, mybir
from concourse.bass_utils import run_bass_kernel_spmd

N = 8192
D = 512
N_CORES = 8
B = N // N_CORES          # 1024 rows per core
P = 128
JC = N // P               # 64 j-chunks
IT = B // 512             # 2 i-halves of the per-core block
ISUB = B // P             # 8 output row-subtiles
KO = D // P               # 4 contraction chunks
SCALE = 0.1
LN_EPS = 1e-6

F32 = mybir.dt.float32
BF16 = mybir.dt.bfloat16
AF = mybir.ActivationFunctionType


def build_kernel(reps=1):
    nc = bacc.Bacc("TRN2", target_bir_lowering=False, debug=False,
                   num_devices=N_CORES)

    # ---- I/O ----
    xT = nc.dram_tensor("xT", [D, N], BF16, kind="ExternalInput")       # x.T
    xb = nc.dram_tensor("xb", [N, D], BF16, kind="ExternalInput")       # x bf16
    xTq = nc.dram_tensor("xTq", [D, B], BF16, kind="ExternalInput")     # x.T own cols
    xq = nc.dram_tensor("xq", [B, D], F32, kind="ExternalInput")        # own rows fp32
    gamma = nc.dram_tensor("gamma", [D], F32, kind="ExternalInput")
    beta = nc.dram_tensor("beta", [D], F32, kind="ExternalInput")
    out = nc.dram_tensor("out", [B, D], F32, kind="ExternalOutput")

    xT_v = xT.ap().rearrange("(ko p) j -> p ko j", p=P)       # [128, 4, 8192]
    xTq_v = xTq.ap().rearrange("(ko p) i -> p ko i", p=P)     # [128, 4, 1024]
    xb_v = xb.ap().rearrange("(c p) d -> p c d", p=P)         # [128, 64, 512]
    xq_v = xq.ap().rearrange("(c p) d -> p c d", p=P)         # [128, 8, 512]

    with tile.TileContext(nc) as tc:
        for rep in range(reps):
            _body(nc, tc, xT_v, xTq_v, xb_v, xq_v, gamma, beta, out, sfx=f"r{rep}")
    nc.compile()
    return nc


def _body(nc, tc, xT_v, xTq_v, xb_v, xq_v, gamma, beta, out, sfx="", ablate=()):
    from contextlib import ExitStack
    ablate = set(ablate)
    ctx = ExitStack()
    with ctx:
        persist = ctx.enter_context(tc.tile_pool(name=f"persist{sfx}", bufs=1))
        small = ctx.enter_context(tc.tile_pool(name=f"small{sfx}", bufs=2))
        stream = ctx.enter_context(tc.tile_pool(name=f"stream{sfx}", bufs=3))
        etile = ctx.enter_context(tc.tile_pool(name=f"etile{sfx}", bufs=3))
        pwork = ctx.enter_context(tc.tile_pool(name=f"pwork{sfx}", bufs=2))
        lnw = ctx.enter_context(tc.tile_pool(name=f"lnw{sfx}", bufs=2))
        dram = ctx.enter_context(tc.tile_pool(name=f"dram{sfx}", bufs=1, space="DRAM"))

        # ---------- load persistent operands ----------
        xTq_sb = persist.tile([P, KO, B], BF16)       # 1 MB
        nc.gpsimd.dma_start(xTq_sb[:], xTq_v[:])

        gamma_b = persist.tile([P, D], F32)
        nc.gpsimd.dma_start(gamma_b[:], bass.AP(tensor=gamma, offset=0,
                                                ap=[[0, P], [1, D]]))
        beta_b = persist.tile([P, D], F32)
        nc.gpsimd.dma_start(beta_b[:], bass.AP(tensor=beta, offset=0,
                                               ap=[[0, P], [1, D]]))

        # ---------- inverse norms ----------
        # invn of own rows first (gates phase A rhs), from fp32
        xq_sb = persist.tile([P, ISUB, D], F32)       # own rows fp32 (2 MB)
        nc.gpsimd.dma_start(xq_sb[:], xq_v[:])
        sq_scr = small.tile([P, D], BF16, tag="sqscr")
        ss_q = persist.tile([P, ISUB], F32)
        for t in range(ISUB):
            nc.scalar.activation(out=sq_scr[:], in_=xq_sb[:, t, :], func=AF.Square,
                                 accum_out=ss_q[:, t:t + 1])
        n_q = small.tile([P, ISUB], F32, tag="nq")
        nc.scalar.activation(out=n_q[:], in_=ss_q[:], func=AF.Sqrt)
        invn_q = small.tile([P, ISUB], F32, tag="invnq")
        nc.vector.reciprocal(out=invn_q[:], in_=n_q[:])
        d_invnq = dram.tile([B], F32)
        nc.gpsimd.dma_start(d_invnq.rearrange("(c p) -> p c", p=P), invn_q[:])
        invnq_b = persist.tile([P, B], F32)
        nc.gpsimd.dma_start(invnq_b[:], bass.AP(tensor=d_invnq.tensor,
                                                offset=d_invnq.offset,
                                                ap=[[0, P], [1, B]]))

        # normalized own columns: xnTq[d, i] = xTq[d, i] * invn_q[i]
        xnTq_sb = persist.tile([P, KO, B], BF16)
        for k in range(KO):
            nc.vector.tensor_tensor(out=xnTq_sb[:, k, :], in0=xTq_sb[:, k, :],
                                    in1=invnq_b[:], op=mybir.AluOpType.mult)

        # invn_all[p, c] = 1/||x_row(c*128+p)|| from bf16 x, in groups of 8
        # chunks so early j-chunks' exp is not gated on the whole pass
        rg = [list(range(N_CORES))]
        ss_all = persist.tile([P, JC], F32)
        n_all = persist.tile([P, JC], F32)
        invn_all = persist.tile([P, JC], F32)
        for g in range(JC // 8):
            xb8 = stream.tile([P, 8, D], BF16, tag="xb8")
            nc.sync.dma_start(xb8[:], xb_v[:, g * 8:(g + 1) * 8, :])
            for c in range(8):
                nc.scalar.activation(out=sq_scr[:], in_=xb8[:, c, :],
                                     func=AF.Square,
                                     accum_out=ss_all[:, g * 8 + c:g * 8 + c + 1])
            nc.scalar.activation(out=n_all[:, g * 8:(g + 1) * 8],
                                 in_=ss_all[:, g * 8:(g + 1) * 8], func=AF.Sqrt)
            nc.vector.reciprocal(out=invn_all[:, g * 8:(g + 1) * 8],
                                 in_=n_all[:, g * 8:(g + 1) * 8])

        # ---------- phase A: E^T tiles + partial row sums ----------
        e_dram = dram.tile([JC, P, B], BF16)          # 16.8 MB scratch
        sacc = persist.tile([P, JC], F32)             # accum_out slots
        if "phase_a" in ablate:
            nc.vector.memset(sacc[:], 1.0)
        with tc.tile_pool(name=f"psum_a{sfx}", bufs=4, space="PSUM") as psum_a:
            for jq in range((JC // 4) if "phase_a" not in ablate else 0):
                xt4 = stream.tile([P, KO, 512], BF16, tag="xt4")
                nc.sync.dma_start(xt4[:], xT_v[:, :, jq * 512:(jq + 1) * 512])
                etq = etile.tile([P, 4, B], BF16, tag="etq")
                for jj in range(4):
                    jc = jq * 4 + jj
                    pt = psum_a.tile([P, B], F32, tag="ph_a")
                    for k in range(KO):
                        for it in range(IT):
                            nc.tensor.matmul(
                                pt[:, it * 512:(it + 1) * 512],
                                xt4[:, k, jj * P:(jj + 1) * P],
                                xnTq_sb[:, k, it * 512:(it + 1) * 512],
                                start=(k == 0), stop=(k == KO - 1))
                    nc.scalar.activation(out=etq[:, jj, :], in_=pt[:],
                                         func=AF.Exp,
                                         scale=invn_all[:, jc:jc + 1],
                                         accum_out=sacc[:, jc:jc + 1])
                nc.sync.dma_start(e_dram[jq * 4:(jq + 1) * 4], etq[:])

        # ---------- collectives: r = global row sums ----------
        c_in = dram.tile([N], F32)
        nc.gpsimd.dma_start(c_in.rearrange("(c p) -> p c", p=P), sacc[:])
        c_ar = dram.tile([N], F32)
        c_rs = dram.tile([B], F32)
        nc.gpsimd.collective_compute("AllReduce", mybir.AluOpType.add,
                                     replica_groups=rg,
                                     ins=[c_in.opt()], outs=[c_ar.opt()])
        nc.gpsimd.collective_compute("ReduceScatter", mybir.AluOpType.add,
                                     replica_groups=rg,
                                     ins=[c_in.opt()], outs=[c_rs.opt()])
        r_all = small.tile([P, JC], F32, tag="rall")
        nc.gpsimd.dma_start(r_all[:], c_ar.rearrange("(c p) -> p c", p=P))
        invr_f = small.tile([P, JC], F32, tag="invrf")
        nc.vector.reciprocal(out=invr_f[:], in_=r_all[:])
        invr_all = persist.tile([P, JC], BF16)
        nc.scalar.copy(out=invr_all[:], in_=invr_f[:])
        rq_b = small.tile([P, B], F32, tag="rqb")
        nc.gpsimd.dma_start(rq_b[:], bass.AP(tensor=c_rs.tensor, offset=c_rs.offset,
                                             ap=[[0, P], [1, B]]))
        invrq_f = small.tile([P, B], F32, tag="invrqf")
        nc.vector.reciprocal(out=invrq_f[:], in_=rq_b[:])
        invrq_b = persist.tile([P, B], BF16)
        nc.scalar.copy(out=invrq_b[:], in_=invrq_f[:])

        # ---------- phase C: x_neg = P^T.T @ x ----------
        psum_c = ctx.enter_context(
            tc.tile_pool(name=f"psum_c{sfx}", bufs=1, space="PSUM"))
        acc = [psum_c.tile([P, D], F32, tag=f"acc{i}", name=f"acc{i}")
               for i in range(ISUB)]
        n_jq = (JC // 4) if "phase_c" not in ablate else 1
        for jq in range(n_jq):
            etq = etile.tile([P, 4, B], BF16, tag="etq")
            nc.sync.dma_start(etq[:], e_dram[jq * 4:(jq + 1) * 4])
            xb4 = stream.tile([P, 4, D], BF16, tag="xb4_c")
            nc.gpsimd.dma_start(xb4[:], xb_v[:, jq * 4:(jq + 1) * 4, :])
            if "mp" in ablate:
                p_t = etq
            else:
                m_t = pwork.tile([P, 4, B], BF16, tag="mt")
                nc.vector.tensor_tensor(
                    out=m_t[:],
                    in0=invrq_b[:, None, :].to_broadcast((P, 4, B)),
                    in1=invr_all[:, jq * 4:(jq + 1) * 4, None].to_broadcast(
                        (P, 4, B)),
                    op=mybir.AluOpType.add)
                p_t = pwork.tile([P, 4, B], BF16, tag="pt")
                nc.vector.tensor_tensor(out=p_t[:], in0=etq[:], in1=m_t[:],
                                        op=mybir.AluOpType.mult)
            for jj in range(4):
                last = (jq == n_jq - 1) and (jj == 3)
                for i in range(ISUB):
                    nc.tensor.matmul(acc[i][:],
                                     p_t[:, jj, i * P:(i + 1) * P],
                                     xb4[:, jj, :],
                                     start=(jq == 0 and jj == 0), stop=last)

        # ---------- tail: y = xq - 0.1*x_neg ; LayerNorm ----------
        eps_t = small.tile([P, 1], F32, tag="eps")
        nc.vector.memset(eps_t[:], LN_EPS)
        y_all = persist.tile([P, ISUB, D], F32)
        mv_all = persist.tile([P, ISUB, 2], F32)
        for i in range(ISUB):
            nc.vector.tensor_scalar(out=y_all[:, i, :], in0=acc[i][:],
                                    scalar1=-SCALE,
                                    scalar2=None, op0=mybir.AluOpType.mult)
            nc.vector.tensor_tensor(out=y_all[:, i, :], in0=y_all[:, i, :],
                                    in1=xq_sb[:, i, :], op=mybir.AluOpType.add)
            stats = lnw.tile([P, 6], F32, tag="stats")
            nc.vector.bn_stats(out=stats[:], in_=y_all[:, i, :])
            nc.vector.bn_aggr(out=mv_all[:, i, :], in_=stats[:])
        std_all = small.tile([P, ISUB], F32, tag="stdall")
        nc.scalar.activation(out=std_all[:], in_=mv_all[:, :, 1], func=AF.Sqrt,
                             bias=eps_t[:])
        rstd_all = small.tile([P, ISUB], F32, tag="rstdall")
        nc.vector.reciprocal(out=rstd_all[:], in_=std_all[:])
        o_t = persist.tile([P, ISUB, D], F32)
        for i in range(ISUB):
            nc.vector.tensor_scalar(out=o_t[:, i, :], in0=y_all[:, i, :],
                                    scalar1=mv_all[:, i, 0:1],
                                    scalar2=rstd_all[:, i:i + 1],
                                    op0=mybir.AluOpType.subtract,
                                    op1=mybir.AluOpType.mult)
            nc.vector.tensor_tensor(out=o_t[:, i, :], in0=o_t[:, i, :],
                                    in1=gamma_b[:], op=mybir.AluOpType.mult)
            nc.vector.tensor_tensor(out=o_t[:, i, :], in0=o_t[:, i, :],
                                    in1=beta_b[:], op=mybir.AluOpType.add)
        nc.sync.dma_start(out.ap().rearrange("(c p) d -> p c d", p=P), o_t[:])


_CACHE = {}


def _get_nc():
    if "nc" not in _CACHE:
        _CACHE["nc"] = build_kernel()
    return _CACHE["nc"]


def make_in_maps(x, gamma, beta):
    x = np.asarray(x, dtype=np.float32)
    xT_bf = np.ascontiguousarray(x.T).astype(ml_dtypes.bfloat16)
    xb_bf = x.astype(ml_dtypes.bfloat16)
    gamma = np.asarray(gamma, dtype=np.float32)
    beta = np.asarray(beta, dtype=np.float32)
    in_maps = []
    for q in range(N_CORES):
        sl = slice(q * B, (q + 1) * B)
        in_maps.append({
            "xT": xT_bf,
            "xb": xb_bf,
            "xTq": np.ascontiguousarray(xT_bf[:, sl]),
            "xq": np.ascontiguousarray(x[sl]),
            "gamma": gamma,
            "beta": beta,
        })
    return in_maps


def kernel(x, gamma, beta):
    nc = _get_nc()
    in_maps = make_in_maps(x, gamma, beta)
    res = run_bass_kernel_spmd(nc, in_maps, core_ids=list(range(N_CORES)))
    out = np.concatenate([res.results[q]["out"] for q in range(N_CORES)], axis=0)
    return out.astype(np.float32)


if __name__ == "__main__":
    rng = np.random.default_rng(0)
    x = rng.standard_normal((N, D), dtype=np.float32)
    gamma = np.ones(D, np.float32)
    beta = np.zeros(D, np.float32)
    o = kernel(x, gamma, beta)
    print("out", o.shape, o.dtype, float(np.abs(o).mean()))



# revision 8
# speedup vs baseline: 1.4147x; 1.4147x over previous
"""ContraNorm kernel for 8x Trainium2 NeuronCores (Bass/Tile), fp8 edition.

Computes, for x [8192, 512] fp32 (gamma/beta [512]):
    xn  = x / max(||x||_row, eps)
    sim = xn @ xn.T
    sim = softmax(sim, axis=1) + softmax(sim, axis=0)
    y   = x - 0.1 * (sim @ x)
    out = LayerNorm(y) * gamma + beta          (eps = 1e-6)

Key structure (per core q, which owns rows I_q = [q*1024, (q+1)*1024)):
  * E = exp(sim) is symmetric, so column sums equal row sums:
        sim' = E * (1/r_i + 1/r_j)  elementwise, r = E @ 1.
  * Phase A: E^T tiles [j on partitions, own i on free] via fp8 DoubleRow
    matmuls: lhsT = x^T (raw, fp8), rhs = 16*xn^T own cols (fp8); ACT Exp
    applies the 1/(16*||x_j||) scale per partition and accumulates partial
    column sums. E stays in SBUF as fp8 (64KB/partition) -- no DRAM spill.
  * Own inverse norms are computed locally from fp32 own rows and
    AllGathered (4KB), instead of every core re-deriving all 8192 norms.
    1/sqrt is computed as exp(-0.5*ln(.)) so the scalar engine stays on a
    single activation table set (natural_log_exp) for the whole kernel.
  * r: AllReduce(32KB) of partial column sums + ReduceScatter for the own
    slice (baseline scheme, avoids core-id branching).
  * Phase C: p = E * (S/r_i + S/r_j) in ONE fused scalar_tensor_tensor op
    per chunk (split across Vector and GpSimd engines), output fp8
    (S = 4096 keeps p in fp8's normal range); x_neg accumulated with fp8
    DoubleRow matmuls against fp8 x rows; the final y folds in the 1/S.
"""

import sys

if "/opt/trn_rl_repo" not in sys.path:
    sys.path.insert(0, "/opt/trn_rl_repo")

import math

import ml_dtypes
import numpy as np

import concourse.bass as bass
import concourse.tile as tile
from concourse import bacc, mybir
from concourse.bass_utils import run_bass_kernel_spmd

N = 8192
D = 512
N_CORES = 8
B = N // N_CORES          # 1024 rows per core
P = 128
JC = N // P               # 64 j-chunks
ISUB = B // P             # 8 output row-subtiles
KO = D // P               # 4 contraction chunks
SCALE = 0.1
S = 4096.0                # p_t pre-scale so fp8 stays in normal range
XN_S = 16.0               # xn pre-scale for fp8 quantization
LN_EPS = 1e-6

F32 = mybir.dt.float32
BF16 = mybir.dt.bfloat16
FP8 = mybir.dt.float8e4
AF = mybir.ActivationFunctionType
DR = mybir.MatmulPerfMode.DoubleRow


def build_kernel(reps=1, ablate=()):
    nc = bacc.Bacc("TRN2", target_bir_lowering=False, debug=False,
                   num_devices=N_CORES)

    # ---- I/O ----
    xT8 = nc.dram_tensor("xT8", [D, N], FP8, kind="ExternalInput")     # x.T fp8
    xb8 = nc.dram_tensor("xb8", [N, D], FP8, kind="ExternalInput")     # x fp8
    xTq = nc.dram_tensor("xTq", [D, B], BF16, kind="ExternalInput")    # own cols
    xq = nc.dram_tensor("xq", [B, D], F32, kind="ExternalInput")       # own rows
    gamma = nc.dram_tensor("gamma", [D], F32, kind="ExternalInput")
    beta = nc.dram_tensor("beta", [D], F32, kind="ExternalInput")
    out = nc.dram_tensor("out", [B, D], F32, kind="ExternalOutput")

    xT8_v = xT8.ap().rearrange("(ko p) j -> p ko j", p=P)     # [128, 4, 8192]
    xb8_v = xb8.ap().rearrange("(c p) d -> p c d", p=P)       # [128, 64, 512]
    xTq_v = xTq.ap().rearrange("(ko p) i -> p ko i", p=P)     # [128, 4, 1024]
    xq_v = xq.ap().rearrange("(c p) d -> p c d", p=P)         # [128, 8, 512]

    with tile.TileContext(nc) as tc:
        for rep in range(reps):
            _body(nc, tc, xT8_v, xb8_v, xTq_v, xq_v, gamma, beta, out,
                  sfx=f"r{rep}", ablate=ablate)
    nc.compile()
    return nc


def _body(nc, tc, xT8_v, xb8_v, xTq_v, xq_v, gamma, beta, out, sfx="",
          ablate=()):
    from contextlib import ExitStack
    ablate = set(ablate)
    ctx = ExitStack()
    with ctx:
        persist = ctx.enter_context(tc.tile_pool(name=f"persist{sfx}", bufs=1))
        small = ctx.enter_context(tc.tile_pool(name=f"small{sfx}", bufs=2))
        ptp = ctx.enter_context(tc.tile_pool(name=f"ptp{sfx}", bufs=3))
        lnw = ctx.enter_context(tc.tile_pool(name=f"lnw{sfx}", bufs=2))
        dram = ctx.enter_context(tc.tile_pool(name=f"dram{sfx}", bufs=1,
                                              space="DRAM"))
        rg = [list(range(N_CORES))]

        # ---------- persistent operands ----------
        xq_sb = persist.tile([P, ISUB, D], F32)       # own rows fp32 (16K/par)
        nc.gpsimd.dma_start(xq_sb[:], xq_v[:])
        xTq_sb = persist.tile([P, KO, B], BF16)       # own cols bf16 (8K/par)
        nc.gpsimd.dma_start(xTq_sb[:], xTq_v[:])
        xT8_sb = persist.tile([P, KO, N], FP8)        # full x.T fp8 (32K/par)
        nc.sync.dma_start(xT8_sb[:], xT8_v[:])
        xb8_sb = persist.tile([P, JC, D], FP8)        # full x fp8 (32K/par)
        nc.sync.dma_start(xb8_sb[:], xb8_v[:])
        gamma_b = persist.tile([P, D], F32)
        nc.gpsimd.dma_start(gamma_b[:], bass.AP(tensor=gamma, offset=0,
                                                ap=[[0, P], [1, D]]))
        beta_b = persist.tile([P, D], F32)
        nc.gpsimd.dma_start(beta_b[:], bass.AP(tensor=beta, offset=0,
                                               ap=[[0, P], [1, D]]))

        # ---------- own inverse norms (vector engine squares) ----------
        sq_scr = small.tile([P, D], BF16, tag="sqscr")
        ss_q = persist.tile([P, ISUB], F32)
        for t in range(ISUB):
            nc.vector.scalar_tensor_tensor(
                out=sq_scr[:], in0=xq_sb[:, t, :], scalar=1.0,
                in1=xq_sb[:, t, :], op0=mybir.AluOpType.mult,
                op1=mybir.AluOpType.mult, accum_out=ss_q[:, t:t + 1])
        ln_ss = small.tile([P, ISUB], F32, tag="lnss")
        nc.scalar.activation(out=ln_ss[:], in_=ss_q[:], func=AF.Ln)
        invn_q = small.tile([P, ISUB], F32, tag="invnq")    # 1/||x_i||
        nc.scalar.activation(out=invn_q[:], in_=ln_ss[:], func=AF.Exp,
                             scale=-0.5)
        ln16_t = small.tile([P, 1], F32, tag="ln16")
        nc.vector.memset(ln16_t[:], math.log(XN_S))
        invn16_q = small.tile([P, ISUB], F32, tag="invn16q")  # 16/||x_i||
        nc.scalar.activation(out=invn16_q[:], in_=ln_ss[:], func=AF.Exp,
                             scale=-0.5, bias=ln16_t[:])

        # share own invn with everyone (4KB AllGather), and broadcast
        # 16/||x_i|| along partitions via a DRAM round-trip
        d_invn = dram.tile([B], F32)
        nc.gpsimd.dma_start(d_invn.rearrange("(c p) -> p c", p=P), invn_q[:])
        g_invn = dram.tile([N], F32)
        nc.gpsimd.collective_compute("AllGather", mybir.AluOpType.bypass,
                                     replica_groups=rg,
                                     ins=[d_invn.opt()], outs=[g_invn.opt()])
        d_invn16 = dram.tile([B], F32)
        nc.gpsimd.dma_start(d_invn16.rearrange("(c p) -> p c", p=P),
                            invn16_q[:])
        invn16_b = persist.tile([P, B], F32)
        nc.gpsimd.dma_start(invn16_b[:],
                            bass.AP(tensor=d_invn16.tensor,
                                    offset=d_invn16.offset,
                                    ap=[[0, P], [1, B]]))

        # normalized own columns, fp8: xnTq[d, i] = xTq[d, i] * 16/||x_i||
        xnTq_sb = persist.tile([P, KO, B], FP8)
        for k in range(KO):
            nc.vector.tensor_tensor(out=xnTq_sb[:, k, :], in0=xTq_sb[:, k, :],
                                    in1=invn16_b[:], op=mybir.AluOpType.mult)

        # all inverse norms, scaled for the exp: 1/(16*||x_j||)
        invn_all = persist.tile([P, JC], F32)
        nc.gpsimd.dma_start(invn_all[:], g_invn.rearrange("(c p) -> p c", p=P))
        invn16r_all = persist.tile([P, JC], F32)
        nc.vector.tensor_scalar(out=invn16r_all[:], in0=invn_all[:],
                                scalar1=1.0 / XN_S, scalar2=None,
                                op0=mybir.AluOpType.mult)

        # ---------- phase A: E^T tiles (SBUF-resident) + partial col sums ---
        e_sb = persist.tile([P, JC, B], FP8)          # 64KB/partition
        sacc = persist.tile([P, JC], F32)
        with tc.tile_pool(name=f"psum_a{sfx}", bufs=3, space="PSUM") as psum_a:
            for jc in range(JC if "phase_a" not in ablate else 0):
                pt = psum_a.tile([P, B], F32, tag="ph_a")
                for kp in range(2):
                    for ih in range(2):
                        nc.tensor.matmul(
                            pt[:, ih * 512:(ih + 1) * 512],
                            xT8_sb[:, 2 * kp:2 * kp + 2,
                                   jc * P:(jc + 1) * P],
                            xnTq_sb[:, 2 * kp:2 * kp + 2,
                                    ih * 512:(ih + 1) * 512],
                            start=(kp == 0), stop=(kp == 1), perf_mode=DR)
                nc.scalar.activation(out=e_sb[:, jc, :], in_=pt[:],
                                     func=AF.Exp,
                                     scale=invn16r_all[:, jc:jc + 1],
                                     accum_out=sacc[:, jc:jc + 1])

        # ---------- collectives: r = global row sums ----------
        c_in = dram.tile([N], F32)
        nc.gpsimd.dma_start(c_in.rearrange("(c p) -> p c", p=P), sacc[:])
        c_ar = dram.tile([N], F32)
        c_rs = dram.tile([B], F32)
        nc.gpsimd.collective_compute("AllReduce", mybir.AluOpType.add,
                                     replica_groups=rg,
                                     ins=[c_in.opt()], outs=[c_ar.opt()])
        nc.gpsimd.collective_compute("ReduceScatter", mybir.AluOpType.add,
                                     replica_groups=rg,
                                     ins=[c_in.opt()], outs=[c_rs.opt()])
        r_all = small.tile([P, JC], F32, tag="rall")
        nc.gpsimd.dma_start(r_all[:], c_ar.rearrange("(c p) -> p c", p=P))
        invr_f = small.tile([P, JC], F32, tag="invrf")
        nc.vector.reciprocal(out=invr_f[:], in_=r_all[:])
        invrS_all = persist.tile([P, JC], F32)        # S / r_j
        nc.vector.tensor_scalar(out=invrS_all[:], in0=invr_f[:], scalar1=S,
                                scalar2=None, op0=mybir.AluOpType.mult)
        rq_s = small.tile([P, ISUB], F32, tag="rqs")
        nc.gpsimd.dma_start(rq_s[:], c_rs.rearrange("(c p) -> p c", p=P))
        invrq_f = small.tile([P, ISUB], F32, tag="invrqf")
        nc.vector.reciprocal(out=invrq_f[:], in_=rq_s[:])
        invrqS_s = small.tile([P, ISUB], F32, tag="invrqSs")
        nc.vector.tensor_scalar(out=invrqS_s[:], in0=invrq_f[:], scalar1=S,
                                scalar2=None, op0=mybir.AluOpType.mult)
        d_invrq = dram.tile([B], F32)
        nc.gpsimd.dma_start(d_invrq.rearrange("(c p) -> p c", p=P),
                            invrqS_s[:])
        invrqS_b = persist.tile([P, B], F32)          # S / r_i, bcast
        nc.gpsimd.dma_start(invrqS_b[:],
                            bass.AP(tensor=d_invrq.tensor,
                                    offset=d_invrq.offset,
                                    ap=[[0, P], [1, B]]))

        # ---------- phase C: x_neg = (E*(S/r_i + S/r_j))^T.T @ x / S -------
        psum_c = ctx.enter_context(
            tc.tile_pool(name=f"psum_c{sfx}", bufs=1, space="PSUM"))
        acc = [psum_c.tile([P, D], F32, tag=f"acc{i}", name=f"acc{i}")
               for i in range(ISUB)]
        npair = (JC // 2) if "phase_c" not in ablate else 1
        for jp in range(npair):
            p_t = ptp.tile([P, 2, B], FP8, tag="pt")
            for h in range(2):
                jc = 2 * jp + h
                if "stt" in ablate:
                    p_t = None
                    break
                nc.vector.scalar_tensor_tensor(
                    out=p_t[:, h, :], in0=invrqS_b[:],
                    scalar=invrS_all[:, jc:jc + 1], in1=e_sb[:, jc, :],
                    op0=mybir.AluOpType.add, op1=mybir.AluOpType.mult)
            src = p_t if p_t is not None else e_sb[:, 2 * jp:2 * jp + 2, :]
            for i in range(ISUB):
                nc.tensor.matmul(acc[i][:],
                                 src[:, :, i * P:(i + 1) * P],
                                 xb8_sb[:, 2 * jp:2 * jp + 2, :],
                                 start=(jp == 0), stop=(jp == npair - 1),
                                 perf_mode=DR)

        # ---------- tail: y = xq - (0.1/S)*acc ; LayerNorm ----------
        # rstd = exp(-0.5 * ln(var + eps)) -- same ACT table set as Exp
        eps_t = small.tile([P, 1], F32, tag="eps")
        nc.vector.memset(eps_t[:], LN_EPS)
        out_v = out.ap().rearrange("(c p) d -> p c d", p=P)
        for i in range(ISUB):
            y_t = lnw.tile([P, D], F32, tag="yt")
            nc.vector.scalar_tensor_tensor(
                out=y_t[:], in0=acc[i][:], scalar=-SCALE / S,
                in1=xq_sb[:, i, :], op0=mybir.AluOpType.mult,
                op1=mybir.AluOpType.add)
            stats = lnw.tile([P, 6], F32, tag="stats")
            nc.vector.bn_stats(out=stats[:], in_=y_t[:])
            mv = lnw.tile([P, 2], F32, tag="mv")
            nc.vector.bn_aggr(out=mv[:], in_=stats[:])
            lnv = lnw.tile([P, 1], F32, tag="lnv")
            nc.scalar.activation(out=lnv[:], in_=mv[:, 1:2], func=AF.Ln,
                                 bias=eps_t[:])
            rstd = lnw.tile([P, 1], F32, tag="rstd")
            nc.scalar.activation(out=rstd[:], in_=lnv[:], func=AF.Exp,
                                 scale=-0.5)
            z_t = lnw.tile([P, D], F32, tag="zt")
            nc.vector.scalar_tensor_tensor(
                out=z_t[:], in0=y_t[:], scalar=mv[:, 0:1],
                in1=gamma_b[:], op0=mybir.AluOpType.subtract,
                op1=mybir.AluOpType.mult)
            o_t = lnw.tile([P, D], F32, tag="ot")
            nc.vector.scalar_tensor_tensor(
                out=o_t[:], in0=z_t[:], scalar=rstd[:],
                in1=beta_b[:], op0=mybir.AluOpType.mult,
                op1=mybir.AluOpType.add)
            nc.sync.dma_start(out_v[:, i, :], o_t[:])


_CACHE = {}


def _get_nc():
    if "nc" not in _CACHE:
        _CACHE["nc"] = build_kernel()
    return _CACHE["nc"]


def make_in_maps(x, gamma, beta):
    x = np.asarray(x, dtype=np.float32)
    f8 = ml_dtypes.float8_e4m3
    xT8 = np.ascontiguousarray(x.T).astype(f8)
    xb8 = x.astype(f8)
    xT_bf = np.ascontiguousarray(x.T).astype(ml_dtypes.bfloat16)
    gamma = np.asarray(gamma, dtype=np.float32)
    beta = np.asarray(beta, dtype=np.float32)
    in_maps = []
    for q in range(N_CORES):
        sl = slice(q * B, (q + 1) * B)
        in_maps.append({
            "xT8": xT8,
            "xb8": xb8,
            "xTq": np.ascontiguousarray(xT_bf[:, sl]),
            "xq": np.ascontiguousarray(x[sl]),
            "gamma": gamma,
            "beta": beta,
        })
    return in_maps


def kernel(x, gamma, beta):
    nc = _get_nc()
    in_maps = make_in_maps(x, gamma, beta)
    res = run_bass_kernel_spmd(nc, in_maps, core_ids=list(range(N_CORES)))
    out = np.concatenate([res.results[q]["out"] for q in range(N_CORES)],
                         axis=0)
    return out.astype(np.float32)


if __name__ == "__main__":
    rng = np.random.default_rng(0)
    x = rng.standard_normal((N, D), dtype=np.float32)
    gamma = np.ones(D, np.float32)
    beta = np.zeros(D, np.float32)
    o = kernel(x, gamma, beta)
    print("out", o.shape, o.dtype, float(np.abs(o).mean()))


# revision 20
# speedup vs baseline: 1.7021x; 1.2031x over previous
"""ContraNorm kernel for 8x Trainium2 NeuronCores (Bass/Tile), fp8 edition.

Computes, for x [8192, 512] fp32 (gamma/beta [512]):
    xn  = x / max(||x||_row, eps)
    sim = xn @ xn.T
    sim = softmax(sim, axis=1) + softmax(sim, axis=0)
    y   = x - 0.1 * (sim @ x)
    out = LayerNorm(y) * gamma + beta          (eps = 1e-6)

Key structure (per core q, which owns rows I_q = [q*1024, (q+1)*1024)):
  * E = exp(sim) is symmetric, so column sums equal row sums:
        sim' = E * (1/r_i + 1/r_j)  elementwise, r = E @ 1.
  * Phase A: E^T tiles [j on partitions, own i on free] via fp8 DoubleRow
    matmuls: lhsT = x^T (raw, fp8), rhs = 16*xn^T own cols (fp8); ACT Exp
    applies the 1/(16*||x_j||) scale per partition and accumulates partial
    column sums. E stays in SBUF as fp8 (64KB/partition) -- no DRAM spill.
  * Own inverse norms are computed locally from fp32 own rows and
    AllGathered (4KB), instead of every core re-deriving all 8192 norms.
    1/sqrt is computed as exp(-0.5*ln(.)) so the scalar engine stays on a
    single activation table set (natural_log_exp) for the whole kernel.
  * r: AllReduce(32KB) of partial column sums + ReduceScatter for the own
    slice (baseline scheme, avoids core-id branching).
  * Phase C: p = E * (S/r_i + S/r_j) in ONE fused scalar_tensor_tensor op
    per chunk (split across Vector and GpSimd engines), output fp8
    (S = 4096 keeps p in fp8's normal range); x_neg accumulated with fp8
    DoubleRow matmuls against fp8 x rows; the final y folds in the 1/S.
"""

import sys

if "/opt/trn_rl_repo" not in sys.path:
    sys.path.insert(0, "/opt/trn_rl_repo")

import math

import ml_dtypes
import numpy as np

import concourse.bass as bass
import concourse.tile as tile
from concourse import bacc, mybir
from concourse.bass_utils import run_bass_kernel_spmd

N = 8192
D = 512
N_CORES = 8
B = N // N_CORES          # 1024 rows per core
P = 128
JC = N // P               # 64 j-chunks
ISUB = B // P             # 8 output row-subtiles
KO = D // P               # 4 contraction chunks
SCALE = 0.1
S = 4096.0                # p_t pre-scale so fp8 stays in normal range
XN_S = 16.0               # xn pre-scale for fp8 quantization
LN_EPS = 1e-6

F32 = mybir.dt.float32
BF16 = mybir.dt.bfloat16
FP8 = mybir.dt.float8e4
AF = mybir.ActivationFunctionType
DR = mybir.MatmulPerfMode.DoubleRow


def build_kernel(reps=1, ablate=()):
    nc = bacc.Bacc("TRN2", target_bir_lowering=False, debug=False,
                   num_devices=N_CORES)

    # ---- I/O ----
    xT8 = nc.dram_tensor("xT8", [D, N], FP8, kind="ExternalInput")     # x.T fp8
    xb8 = nc.dram_tensor("xb8", [N, D], FP8, kind="ExternalInput")     # x fp8
    xTq = nc.dram_tensor("xTq", [D, B], BF16, kind="ExternalInput")    # own cols
    xq = nc.dram_tensor("xq", [B, D], F32, kind="ExternalInput")       # own rows
    gamma = nc.dram_tensor("gamma", [D], F32, kind="ExternalInput")
    beta = nc.dram_tensor("beta", [D], F32, kind="ExternalInput")
    out = nc.dram_tensor("out", [B, D], F32, kind="ExternalOutput")

    xT8_v = xT8.ap().rearrange("(ko p) j -> p ko j", p=P)     # [128, 4, 8192]
    xb8_v = xb8.ap().rearrange("(c p) d -> p c d", p=P)       # [128, 64, 512]
    xTq_v = xTq.ap().rearrange("(ko p) i -> p ko i", p=P)     # [128, 4, 1024]
    xq_v = xq.ap().rearrange("(c p) d -> p c d", p=P)         # [128, 8, 512]

    with tile.TileContext(nc) as tc:
        for rep in range(reps):
            _body(nc, tc, xT8_v, xb8_v, xTq_v, xq_v, gamma, beta, out,
                  sfx=f"r{rep}", ablate=ablate)
    nc.compile()
    return nc


def _body(nc, tc, xT8_v, xb8_v, xTq_v, xq_v, gamma, beta, out, sfx="",
          ablate=()):
    from contextlib import ExitStack
    ablate = set(ablate)
    ctx = ExitStack()
    with ctx:
        persist = ctx.enter_context(tc.tile_pool(name=f"persist{sfx}", bufs=1))
        small = ctx.enter_context(tc.tile_pool(name=f"small{sfx}", bufs=2))
        ptp = ctx.enter_context(tc.tile_pool(name=f"ptp{sfx}", bufs=4))
        lnw = ctx.enter_context(tc.tile_pool(name=f"lnw{sfx}", bufs=2))
        dram = ctx.enter_context(tc.tile_pool(name=f"dram{sfx}", bufs=1,
                                              space="DRAM"))
        rg = [list(range(N_CORES))]

        # early dummy collective: absorbs the cross-core launch stagger
        # while input DMAs stream, so the real collectives later rendezvous
        # against already-aligned cores
        d_dum = dram.tile([8], F32)
        d_dum2 = dram.tile([8], F32)
        if "coll" not in ablate:
            nc.gpsimd.collective_compute("AllReduce", mybir.AluOpType.add,
                                         replica_groups=rg,
                                         ins=[d_dum.opt()], outs=[d_dum2.opt()])

        # ---------- persistent operands ----------
        xq_sb = persist.tile([P, ISUB, D], F32)       # own rows fp32 (16K/par)
        nc.gpsimd.dma_start(xq_sb[:], xq_v[:])
        xTq_sb = persist.tile([P, KO, B], BF16)       # own cols bf16 (8K/par)
        nc.gpsimd.dma_start(xTq_sb[:], xTq_v[:])
        xT8_sb = persist.tile([P, KO, N], FP8)        # full x.T fp8 (32K/par)
        nc.sync.dma_start(xT8_sb[:, :, 0:N // 2], xT8_v[:, :, 0:N // 2])
        nc.sync.dma_start(xT8_sb[:, :, N // 2:N], xT8_v[:, :, N // 2:N])
        xb8_sb = persist.tile([P, JC, D], FP8)        # full x fp8 (32K/par)
        nc.sync.dma_start(xb8_sb[:], xb8_v[:])
        gamma_b = persist.tile([P, D], F32)
        nc.gpsimd.dma_start(gamma_b[:], bass.AP(tensor=gamma, offset=0,
                                                ap=[[0, P], [1, D]]))
        beta_b = persist.tile([P, D], F32)
        nc.gpsimd.dma_start(beta_b[:], bass.AP(tensor=beta, offset=0,
                                               ap=[[0, P], [1, D]]))

        # ---------- own inverse norms (vector engine squares) ----------
        sq_scr = small.tile([P, D], BF16, tag="sqscr")
        ss_q = persist.tile([P, ISUB], F32)
        for t in range(ISUB):
            nc.vector.scalar_tensor_tensor(
                out=sq_scr[:], in0=xq_sb[:, t, :], scalar=1.0,
                in1=xq_sb[:, t, :], op0=mybir.AluOpType.mult,
                op1=mybir.AluOpType.mult, accum_out=ss_q[:, t:t + 1])
        ln_ss = small.tile([P, ISUB], F32, tag="lnss")
        nc.scalar.activation(out=ln_ss[:], in_=ss_q[:], func=AF.Ln)
        invn_q = small.tile([P, ISUB], F32, tag="invnq")    # 1/||x_i||
        nc.scalar.activation(out=invn_q[:], in_=ln_ss[:], func=AF.Exp,
                             scale=-0.5)
        ln16_t = small.tile([P, 1], F32, tag="ln16")
        nc.vector.memset(ln16_t[:], math.log(XN_S))
        invn16_q = small.tile([P, ISUB], F32, tag="invn16q")  # 16/||x_i||
        nc.scalar.activation(out=invn16_q[:], in_=ln_ss[:], func=AF.Exp,
                             scale=-0.5, bias=ln16_t[:])

        # share own invn with everyone (4KB AllGather), and broadcast
        # 16/||x_i|| along partitions via a DRAM round-trip
        d_invn = dram.tile([B], F32)
        nc.gpsimd.dma_start(d_invn.rearrange("(c p) -> p c", p=P), invn_q[:])
        g_invn = dram.tile([N], F32)
        if "coll" not in ablate and "no_ag" not in ablate:
            nc.gpsimd.collective_compute("AllGather", mybir.AluOpType.bypass,
                                         replica_groups=rg,
                                         ins=[d_invn.opt()],
                                         outs=[g_invn.opt()])
        d_invn16 = dram.tile([B], F32)
        nc.gpsimd.dma_start(d_invn16.rearrange("(c p) -> p c", p=P),
                            invn16_q[:])
        invn16_b = persist.tile([P, B], F32)
        nc.gpsimd.dma_start(invn16_b[:],
                            bass.AP(tensor=d_invn16.tensor,
                                    offset=d_invn16.offset,
                                    ap=[[0, P], [1, B]]))

        # normalized own columns, fp8: xnTq[d, i] = xTq[d, i] * 16/||x_i||
        xnTq_sb = persist.tile([P, KO, B], FP8)
        for k in range(KO):
            nc.vector.tensor_tensor(out=xnTq_sb[:, k, :], in0=xTq_sb[:, k, :],
                                    in1=invn16_b[:], op=mybir.AluOpType.mult)

        # all inverse norms, scaled for the exp: 1/(16*||x_j||)
        invn_all = persist.tile([P, JC], F32)
        nc.gpsimd.dma_start(invn_all[:], g_invn.rearrange("(c p) -> p c", p=P))
        invn16r_all = persist.tile([P, JC], F32)
        nc.vector.tensor_scalar(out=invn16r_all[:], in0=invn_all[:],
                                scalar1=1.0 / XN_S, scalar2=None,
                                op0=mybir.AluOpType.mult)

        # ---------- phase A: E^T tiles (SBUF-resident) + partial col sums ---
        e_sb = persist.tile([P, JC, B], FP8)          # 64KB/partition
        sacc = persist.tile([P, JC], F32)
        if "phase_a" in ablate:
            nc.vector.memset(sacc[:], 1.0)
            for jc in range(JC):
                nc.vector.memset(e_sb[:, jc, :], 1.0)
        with tc.tile_pool(name=f"psum_a{sfx}", bufs=3, space="PSUM") as psum_a:
            for jc in range(JC if "phase_a" not in ablate else 0):
                pt = psum_a.tile([P, B], F32, tag="ph_a")
                nkp = 1 if "amm2" in ablate else 2
                for kp in range(nkp):
                    for ih in range(2):
                        nc.tensor.matmul(
                            pt[:, ih * 512:(ih + 1) * 512],
                            xT8_sb[:, 2 * kp:2 * kp + 2,
                                   jc * P:(jc + 1) * P],
                            xnTq_sb[:, 2 * kp:2 * kp + 2,
                                    ih * 512:(ih + 1) * 512],
                            start=(kp == 0), stop=(kp == nkp - 1),
                            perf_mode=DR)
                if "noexp" in ablate:
                    nc.vector.tensor_scalar(
                        out=e_sb[:, jc, :], in0=pt[:], scalar1=1.0,
                        scalar2=None, op0=mybir.AluOpType.mult,
                        accum_out=sacc[:, jc:jc + 1])
                else:
                    nc.scalar.activation(out=e_sb[:, jc, :], in_=pt[:],
                                         func=AF.Exp,
                                         scale=invn16r_all[:, jc:jc + 1],
                                         accum_out=sacc[:, jc:jc + 1])

        # ---------- collectives: r = global row sums ----------
        c_in = dram.tile([N], F32)
        nc.gpsimd.dma_start(c_in.rearrange("(c p) -> p c", p=P), sacc[:])
        c_ar = dram.tile([N], F32)
        c_rs = dram.tile([B], F32)
        if "coll" not in ablate and "no_ar" not in ablate:
            nc.gpsimd.collective_compute("AllReduce", mybir.AluOpType.add,
                                         replica_groups=rg,
                                         ins=[c_in.opt()], outs=[c_ar.opt()])
            if "no_rs" not in ablate:
                nc.gpsimd.collective_compute("ReduceScatter",
                                             mybir.AluOpType.add,
                                             replica_groups=rg,
                                             ins=[c_in.opt()],
                                             outs=[c_rs.opt()])
        r_all = small.tile([P, JC], F32, tag="rall")
        nc.gpsimd.dma_start(r_all[:], c_ar.rearrange("(c p) -> p c", p=P))
        invr_f = small.tile([P, JC], F32, tag="invrf")
        nc.vector.reciprocal(out=invr_f[:], in_=r_all[:])
        invrS_all = persist.tile([P, JC], F32)        # S / r_j
        nc.vector.tensor_scalar(out=invrS_all[:], in0=invr_f[:], scalar1=S,
                                scalar2=None, op0=mybir.AluOpType.mult)
        rq_s = small.tile([P, ISUB], F32, tag="rqs")
        nc.gpsimd.dma_start(rq_s[:], c_rs.rearrange("(c p) -> p c", p=P))
        invrq_f = small.tile([P, ISUB], F32, tag="invrqf")
        nc.vector.reciprocal(out=invrq_f[:], in_=rq_s[:])
        invrqS_s = small.tile([P, ISUB], BF16, tag="invrqSs")
        nc.vector.tensor_scalar(out=invrqS_s[:], in0=invrq_f[:], scalar1=S,
                                scalar2=None, op0=mybir.AluOpType.mult)
        d_invrq = dram.tile([B], BF16)
        nc.gpsimd.dma_start(d_invrq.rearrange("(c p) -> p c", p=P),
                            invrqS_s[:])
        invrqS_b = persist.tile([P, B], BF16)         # S / r_i, bcast
        nc.gpsimd.dma_start(invrqS_b[:],
                            bass.AP(tensor=d_invrq.tensor,
                                    offset=d_invrq.offset,
                                    ap=[[0, P], [1, B]]))

        # ---------- phase C: x_neg = (E*(S/r_i + S/r_j))^T.T @ x / S -------
        psum_c = ctx.enter_context(
            tc.tile_pool(name=f"psum_c{sfx}", bufs=1, space="PSUM"))
        acc = [psum_c.tile([P, D], F32, tag=f"acc{i}", name=f"acc{i}")
               for i in range(ISUB)]
        npair = (JC // 2) if "phase_c" not in ablate else 1
        for jp in range(npair):
            p_t = ptp.tile([P, 2, B], FP8, tag="pt")
            for h in range(2):
                jc = 2 * jp + h
                if "stt" in ablate:
                    p_t = None
                    break
                nc.vector.scalar_tensor_tensor(
                    out=p_t[:, h, :], in0=invrqS_b[:],
                    scalar=invrS_all[:, jc:jc + 1], in1=e_sb[:, jc, :],
                    op0=mybir.AluOpType.add, op1=mybir.AluOpType.mult)
            src = p_t if p_t is not None else e_sb[:, 2 * jp:2 * jp + 2, :]
            for i in range(ISUB):
                nc.tensor.matmul(acc[i][:],
                                 src[:, :, i * P:(i + 1) * P],
                                 xb8_sb[:, 2 * jp:2 * jp + 2, :],
                                 start=(jp == 0), stop=(jp == npair - 1),
                                 perf_mode=DR)

        # ---------- tail: y = xq - (0.1/S)*acc ; LayerNorm ----------
        # rstd = exp(-0.5 * ln(var + eps)) -- same ACT table set as Exp
        eps_t = small.tile([P, 1], F32, tag="eps")
        nc.vector.memset(eps_t[:], LN_EPS)
        out_v = out.ap().rearrange("(c p) d -> p c d", p=P)
        for i in range(ISUB if "tail" not in ablate else 0):
            y_t = lnw.tile([P, D], F32, tag="yt")
            nc.vector.scalar_tensor_tensor(
                out=y_t[:], in0=acc[i][:], scalar=-SCALE / S,
                in1=xq_sb[:, i, :], op0=mybir.AluOpType.mult,
                op1=mybir.AluOpType.add)
            stats = lnw.tile([P, 6], F32, tag="stats")
            nc.vector.bn_stats(out=stats[:], in_=y_t[:])
            mv = lnw.tile([P, 2], F32, tag="mv")
            nc.vector.bn_aggr(out=mv[:], in_=stats[:])
            lnv = lnw.tile([P, 1], F32, tag="lnv")
            nc.scalar.activation(out=lnv[:], in_=mv[:, 1:2], func=AF.Ln,
                                 bias=eps_t[:])
            rstd = lnw.tile([P, 1], F32, tag="rstd")
            nc.scalar.activation(out=rstd[:], in_=lnv[:], func=AF.Exp,
                                 scale=-0.5)
            z_t = lnw.tile([P, D], F32, tag="zt")
            nc.vector.scalar_tensor_tensor(
                out=z_t[:], in0=y_t[:], scalar=mv[:, 0:1],
                in1=gamma_b[:], op0=mybir.AluOpType.subtract,
                op1=mybir.AluOpType.mult)
            o_t = lnw.tile([P, D], F32, tag="ot")
            nc.vector.scalar_tensor_tensor(
                out=o_t[:], in0=z_t[:], scalar=rstd[:],
                in1=beta_b[:], op0=mybir.AluOpType.mult,
                op1=mybir.AluOpType.add)
            nc.sync.dma_start(out_v[:, i, :], o_t[:])


_CACHE = {}


def _get_nc():
    if "nc" not in _CACHE:
        _CACHE["nc"] = build_kernel()
    return _CACHE["nc"]


def make_in_maps(x, gamma, beta):
    x = np.asarray(x, dtype=np.float32)
    f8 = ml_dtypes.float8_e4m3
    xT8 = np.ascontiguousarray(x.T).astype(f8)
    xb8 = x.astype(f8)
    xT_bf = np.ascontiguousarray(x.T).astype(ml_dtypes.bfloat16)
    gamma = np.asarray(gamma, dtype=np.float32)
    beta = np.asarray(beta, dtype=np.float32)
    in_maps = []
    for q in range(N_CORES):
        sl = slice(q * B, (q + 1) * B)
        in_maps.append({
            "xT8": xT8,
            "xb8": xb8,
            "xTq": np.ascontiguousarray(xT_bf[:, sl]),
            "xq": np.ascontiguousarray(x[sl]),
            "gamma": gamma,
            "beta": beta,
        })
    return in_maps


def kernel(x, gamma, beta):
    nc = _get_nc()
    in_maps = make_in_maps(x, gamma, beta)
    res = run_bass_kernel_spmd(nc, in_maps, core_ids=list(range(N_CORES)))
    out = np.concatenate([res.results[q]["out"] for q in range(N_CORES)],
                         axis=0)
    return out.astype(np.float32)


if __name__ == "__main__":
    rng = np.random.default_rng(0)
    x = rng.standard_normal((N, D), dtype=np.float32)
    gamma = np.ones(D, np.float32)
    beta = np.zeros(D, np.float32)
    o = kernel(x, gamma, beta)
    print("out", o.shape, o.dtype, float(np.abs(o).mean()))
